# revision 6
# baseline (speedup 1.0000x reference)
"""Trainium2 Bass kernel for the combined loss (KL + CE + InfoNCE + focal + adv CE).

Strategy (8 NeuronCores, v2 — symmetric half-band InfoNCE):
  - o / master / o_adv sharded by rows (512/core), shipped bf16.
  - InfoNCE: feats = concat(feat_pooled, feat_pooled_masked) -> [8192, 256],
    transposed to [256, 8192] bf16 with per-core column roll (core's 1024 query
    rows at rolled block positions {0..3, 32..35}).  Exploits G = G^T: each
    query block computes only the half-open circulant band of col-blocks
    (+1..+32), with triangular masks on the self block and the +32 block so
    every unordered pair is computed exactly once.  Each computed exp(l_ij)
    serves row i via the ScalarE accumulate (row sums) and row j via a GpSimd
    partition all-reduce over the exp tile (column sums).  Per-core partial
    row/col sums + positive-pair traces go back to the host, which assembles
    the full 8192 row sums, takes the log, and averages (the only cross-core
    reduction in the algorithm).
  - Normalization: squares on DVE (bf16 2x), column sums-of-squares via a
    bf16 ones-matmul (broadcast across partitions), 1/sqrt on ScalarE Rsqrt,
    normalize multiply on DVE -- keeps the big ACT passes for exp only.
"""

import numpy as np
import ml_dtypes

import concourse.bacc as bacc
import concourse.tile as tile
from concourse import mybir
from concourse import bass_isa
from concourse.bass_utils import run_bass_kernel_spmd

F32 = mybir.dt.float32
BF16 = mybir.dt.bfloat16
FP16 = mybir.dt.float16
AF = mybir.ActivationFunctionType
ALU = mybir.AluOpType
AX = mybir.AxisListType

NCORES = 8
B, C, D = 4096, 1000, 256
RB = B // NCORES          # 512 rows of the [B, C] tensors per core
NT = RB // 128            # 4 row-tiles per core
N2 = 2 * B                # 8192 infoNCE rows
NBLK = N2 // 128          # 64 column blocks
QBLKS = [0, 1, 2, 3, 32, 33, 34, 35]   # rolled block positions of queries
NQ = len(QBLKS)
BANDC = 32 * 128          # 4096 band columns per query block
ESW = BANDC + 128         # es tile width: band + self block
KCH = 1024                # gram / psum chunk width
SUB = 512                 # matmul moving free dim

KL_TEMP = 4.0
KL_INTERP = 0.5
NCE_TEMP = 0.07
NEG_BIG = -1.0e9


def _col_runs(start, e0, e1):
    """Contiguous rolled-column runs covering band offsets [e0, e1)."""
    runs = []
    e = e0
    while e < e1:
        rs = (start + e) % N2
        ln = min(e1 - e, N2 - rs)
        runs.append((rs, e, ln))
        e += ln
    return runs


def _build_module():
    nc = bacc.Bacc("TRN2", target_bir_lowering=False, debug=False)

    o_d = nc.dram_tensor("o", [RB, C], BF16, kind="ExternalInput")
    m_d = nc.dram_tensor("m", [RB, C], BF16, kind="ExternalInput")
    a_d = nc.dram_tensor("a", [RB, C], BF16, kind="ExternalInput")
    tg_d = nc.dram_tensor("tg", [128, NT], FP16, kind="ExternalInput")
    ta_d = nc.dram_tensor("ta", [128, NT], FP16, kind="ExternalInput")
    ft_d = nc.dram_tensor("ft", [256, N2], BF16, kind="ExternalInput")
    res_d = nc.dram_tensor("res", [8, 1], F32, kind="ExternalOutput")
    rsp_d = nc.dram_tensor("rsp", [128, NQ * 5], F32, kind="ExternalOutput")
    csp_d = nc.dram_tensor("csp", [NQ, ESW], F32, kind="ExternalOutput")

    iota_np = np.tile(np.arange(C, dtype=np.float16), (128, 1))
    ident_np = np.eye(128, dtype=np.float32)
    identb_np = np.eye(128).astype(ml_dtypes.bfloat16)
    onesb_np = np.ones((128, 128)).astype(ml_dtypes.bfloat16)
    onesf_np = np.ones((128, 1), dtype=np.float32)
    r_idx = np.arange(128)[:, None]
    c_idx = np.arange(128)[None, :]
    # matmul adds lhsT^T @ I = lhsT^T: bake the transpose into the constants.
    # keep c<=r  (kill c>r):  lhsT = NEG*(r>c)
    mask_strict_np = (NEG_BIG * (r_idx > c_idx)).astype(ml_dtypes.bfloat16)
    # keep c<r   (kill c>=r): lhsT = NEG*(r>=c)
    mask_incl_np = (NEG_BIG * (r_idx >= c_idx)).astype(ml_dtypes.bfloat16)

    iota_d = nc.inline_tensor(iota_np, "iota_c")
    ident_d = nc.inline_tensor(ident_np, "ident_c")
    identb_d = nc.inline_tensor(identb_np, "identb_c")
    onesb_d = nc.inline_tensor(onesb_np, "onesb_c")
    onesf_d = nc.inline_tensor(onesf_np, "onesf_c")
    masks_d = nc.inline_tensor(mask_strict_np, "masks_c")
    maski_d = nc.inline_tensor(mask_incl_np, "maski_c")

    from contextlib import ExitStack
    with tile.TileContext(nc) as tc:
        with (
            tc.tile_pool(name="persist", bufs=1) as persist,
            tc.tile_pool(name="io", bufs=2) as iop,
            tc.tile_pool(name="scr", bufs=4) as scrp,
            tc.tile_pool(name="vec", bufs=1) as vecp,
            tc.tile_pool(name="ps", bufs=2, space="PSUM") as psp,
            ExitStack() as late_ctx,
        ):
            dma = nc.default_dma_engine.dma_start

            iota_t = persist.tile([128, C], FP16, tag="iota")
            dma(out=iota_t[:], in_=iota_d[:])
            ident_t = persist.tile([128, 128], F32, tag="ident")
            dma(out=ident_t[:], in_=ident_d[:])
            identb_t = persist.tile([128, 128], BF16, tag="identb")
            dma(out=identb_t[:], in_=identb_d[:])
            onesb_t = persist.tile([128, 128], BF16, tag="onesb")
            dma(out=onesb_t[:], in_=onesb_d[:])
            onesf_t = persist.tile([128, 1], F32, tag="onesf")
            dma(out=onesf_t[:], in_=onesf_d[:])
            masks_t = persist.tile([128, 128], BF16, tag="masks")
            dma(out=masks_t[:], in_=masks_d[:])
            maski_t = persist.tile([128, 128], BF16, tag="maski")
            dma(out=maski_t[:], in_=maski_d[:])
            tg_t = persist.tile([128, NT], FP16, tag="tg")
            dma(out=tg_t[:], in_=tg_d[:])
            ta_t = persist.tile([128, NT], FP16, tag="ta")
            dma(out=ta_t[:], in_=ta_d[:])

            hn0 = persist.tile([128, N2], BF16, tag="hn0")
            hn1 = persist.tile([128, N2], BF16, tag="hn1")
            rsp_t = persist.tile([128, NQ * 5], F32, tag="rsp")
            trace_t = vecp.tile([128, 4], F32, tag="trace")

            # cekl per-row stat slots
            S1 = vecp.tile([128, NT], F32, tag="S1")
            ST = vecp.tile([128, NT], F32, tag="ST")
            SM = vecp.tile([128, NT], F32, tag="SM")
            SA = vecp.tile([128, NT], F32, tag="SA")
            PP = vecp.tile([128, NT], F32, tag="PP")
            GO = vecp.tile([128, NT], F32, tag="GO")
            GA = vecp.tile([128, NT], F32, tag="GA")

            def cekl_tile(t):
                rsl = slice(t * 128, (t + 1) * 128)
                o_t = iop.tile([128, C], BF16, tag="o")
                dma(out=o_t[:], in_=o_d[rsl, :])
                m_t = iop.tile([128, C], BF16, tag="m")
                dma(out=m_t[:], in_=m_d[rsl, :])
                a_t = iop.tile([128, C], BF16, tag="a")
                dma(out=a_t[:], in_=a_d[rsl, :])

                e1 = scrp.tile([128, C], BF16, tag="scr1000")
                nc.scalar.activation(e1[:], o_t[:], AF.Exp, scale=1.0,
                                     accum_out=S1[:, t:t + 1])
                e2 = scrp.tile([128, C], BF16, tag="scr1000")
                nc.scalar.activation(e2[:], o_t[:], AF.Exp,
                                     scale=float(1.0 / KL_TEMP),
                                     accum_out=ST[:, t:t + 1])
                em_t = iop.tile([128, C], BF16, tag="em")
                nc.scalar.activation(em_t[:], m_t[:], AF.Exp,
                                     scale=float(1.0 / KL_TEMP),
                                     accum_out=SM[:, t:t + 1])
                e3 = scrp.tile([128, C], BF16, tag="scr1000")
                nc.scalar.activation(e3[:], a_t[:], AF.Exp, scale=1.0,
                                     accum_out=SA[:, t:t + 1])

                d_t = iop.tile([128, C], BF16, tag="d")
                nc.vector.tensor_sub(d_t[:], m_t[:], o_t[:])
                pr = scrp.tile([128, C], BF16, tag="scr1000")
                nc.vector.scalar_tensor_tensor(
                    out=pr[:], in0=d_t[:], scalar=1.0, in1=em_t[:],
                    op0=ALU.mult, op1=ALU.mult, accum_out=PP[:, t:t + 1])
                g1 = scrp.tile([128, C], BF16, tag="scr1000")
                nc.vector.scalar_tensor_tensor(
                    out=g1[:], in0=iota_t[:], scalar=tg_t[:, t:t + 1],
                    in1=o_t[:], op0=ALU.is_equal, op1=ALU.mult,
                    accum_out=GO[:, t:t + 1])
                g2 = scrp.tile([128, C], BF16, tag="scr1000")
                nc.vector.scalar_tensor_tensor(
                    out=g2[:], in0=iota_t[:], scalar=ta_t[:, t:t + 1],
                    in1=a_t[:], op0=ALU.is_equal, op1=ALU.mult,
                    accum_out=GA[:, t:t + 1])

            # ---- prologue: cekl tile 0 keeps ACT busy during feature DMA ----
            cekl_tile(0)

            raw_pool_cm = tc.tile_pool(name="raw", bufs=1)
            rawp = raw_pool_cm.__enter__()
            h0 = rawp.tile([128, N2], BF16, tag="h0")
            h1 = rawp.tile([128, N2], BF16, tag="h1")
            sq0 = rawp.tile([128, N2], BF16, tag="sq0")
            sq1 = rawp.tile([128, N2], BF16, tag="sq1")
            rn = rawp.tile([128, N2], BF16, tag="rn")
            NCH = N2 // KCH
            for ch in range(NCH):
                sl = slice(ch * KCH, (ch + 1) * KCH)
                dma(out=h0[:, sl], in_=ft_d[0:128, sl])
                dma(out=h1[:, sl], in_=ft_d[128:256, sl])

            cekl_tile(1)

            # ---- normalization pipeline, chunked ----
            for ch in range(NCH):
                sl = slice(ch * KCH, (ch + 1) * KCH)
                nc.vector.tensor_mul(sq0[:, sl], h0[:, sl], h0[:, sl])
                nc.vector.tensor_mul(sq1[:, sl], h1[:, sl], h1[:, sl])
                ps_n = psp.tile([128, KCH], F32, tag="ps")
                for half, sq in ((0, sq0), (1, sq1)):
                    for s in range(KCH // SUB):
                        c0 = ch * KCH + s * SUB
                        osl = slice(s * SUB, (s + 1) * SUB)
                        nc.tensor.matmul(ps_n[:, osl], onesb_t[:],
                                         sq[:, c0:c0 + SUB],
                                         start=(half == 0), stop=(half == 1))
                # rn = s^-0.5 via exp(-0.5*ln(s)); Ln+Exp share one table set
                # (Rsqrt is blocked by bass for accuracy reasons)
                nc.scalar.activation(rn[:, sl], ps_n[:], AF.Ln)
                nc.scalar.activation(rn[:, sl], rn[:, sl], AF.Exp, scale=-0.5)
                nc.vector.tensor_mul(hn0[:, sl], h0[:, sl], rn[:, sl])
                nc.vector.tensor_mul(hn1[:, sl], h1[:, sl], rn[:, sl])

            cekl_tile(2)
            cekl_tile(3)

            # raw features / squares / rn are dead; release before es/cs pools.
            raw_pool_cm.__exit__(None, None, None)
            esp = late_ctx.enter_context(tc.tile_pool(name="es", bufs=3))
            csp = late_ctx.enter_context(tc.tile_pool(name="cs", bufs=2))

            # ---- symmetric half-band InfoNCE ----
            for qi, Bq in enumerate(QBLKS):
                cb = 128 * Bq
                start = 128 * (Bq + 1)
                lhs0 = hn0[:, cb:cb + 128]
                lhs1 = hn1[:, cb:cb + 128]
                es_t = esp.tile([128, ESW], BF16, tag="es")
                for kc in range(4):
                    ps_k = psp.tile([128, KCH], F32, tag="ps")
                    for half, hn, lhsT in ((0, hn0, lhs0), (1, hn1, lhs1)):
                        for s in range(KCH // SUB):
                            base = kc * KCH + s * SUB
                            for (rs, e, ln) in _col_runs(start, base,
                                                         base + SUB):
                                off = e - kc * KCH
                                nc.tensor.matmul(
                                    ps_k[:, off:off + ln], lhsT,
                                    hn[:, rs:rs + ln],
                                    start=(half == 0), stop=(half == 1))
                        if half == 0 and kc == 3:
                            # +32 block triangular mask (last 128 cols)
                            mk = masks_t if qi < 4 else maski_t
                            nc.tensor.matmul(ps_k[:, 896:1024], mk[:],
                                             identb_t[:], start=False,
                                             stop=False,
                                             skip_group_check=True)
                    nc.scalar.activation(
                        es_t[:, kc * KCH:(kc + 1) * KCH], ps_k[:], AF.Exp,
                        scale=float(1.0 / NCE_TEMP),
                        accum_out=rsp_t[:, qi * 5 + kc:qi * 5 + kc + 1])
                    if kc == 3 and qi < 4:
                        tr_scr = scrp.tile([128, 128], F32, tag="trscr")
                        nc.vector.scalar_tensor_tensor(
                            out=tr_scr[:], in0=ps_k[:, 896:1024], scalar=1.0,
                            in1=ident_t[:], op0=ALU.mult, op1=ALU.mult,
                            accum_out=trace_t[:, qi:qi + 1])
                # self block (strict lower kept)
                ps_s = psp.tile([128, 128], F32, tag="pss")
                nc.tensor.matmul(ps_s[:], lhs0, hn0[:, cb:cb + 128],
                                 start=True, stop=False)
                nc.tensor.matmul(ps_s[:], lhs1, hn1[:, cb:cb + 128],
                                 start=False, stop=False)
                nc.tensor.matmul(ps_s[:], maski_t[:], identb_t[:],
                                 start=False, stop=True)
                nc.scalar.activation(
                    es_t[:, BANDC:ESW], ps_s[:], AF.Exp,
                    scale=float(1.0 / NCE_TEMP),
                    accum_out=rsp_t[:, qi * 5 + 4:qi * 5 + 5])
                # column sums of this query block's exp tile
                cs_t = csp.tile([128, ESW], F32, tag="cs")
                nc.gpsimd.partition_all_reduce(cs_t[:], es_t[:], 128,
                                               bass_isa.ReduceOp.add)
                dma(out=csp_d[qi:qi + 1, :], in_=cs_t[0:1, :])

            # ---- epilogue on [128, NT] stat vectors ----
            lse1 = vecp.tile([128, NT], F32, tag="lse1")
            nc.scalar.activation(lse1[:], S1[:], AF.Ln)
            lseT = vecp.tile([128, NT], F32, tag="lseT")
            nc.scalar.activation(lseT[:], ST[:], AF.Ln)
            lsem = vecp.tile([128, NT], F32, tag="lsem")
            nc.scalar.activation(lsem[:], SM[:], AF.Ln)
            lsea = vecp.tile([128, NT], F32, tag="lsea")
            nc.scalar.activation(lsea[:], SA[:], AF.Ln)

            ce = vecp.tile([128, NT], F32, tag="ce")
            nc.vector.tensor_sub(ce[:], lse1[:], GO[:])
            adv = vecp.tile([128, NT], F32, tag="adv")
            nc.vector.tensor_sub(adv[:], lsea[:], GA[:])

            # kl_row = PP/(T*SM) - lsem + lseT
            invSM = vecp.tile([128, NT], F32, tag="invSM")
            nc.vector.reciprocal(invSM[:], SM[:])
            kl = vecp.tile([128, NT], F32, tag="kl")
            nc.vector.tensor_mul(kl[:], PP[:], invSM[:])
            nc.vector.tensor_scalar_mul(kl[:], kl[:], float(1.0 / KL_TEMP))
            nc.vector.tensor_sub(kl[:], kl[:], lsem[:])
            nc.vector.tensor_add(kl[:], kl[:], lseT[:])

            # focal_row = (1-pt)^gamma * ce,  pt = exp(-ce)
            pt = vecp.tile([128, NT], F32, tag="pt")
            nc.scalar.activation(pt[:], ce[:], AF.Exp, scale=-1.0)
            c1 = vecp.tile([128, NT], F32, tag="c1")
            nc.vector.tensor_scalar(c1[:], pt[:], 0.5, None, op0=ALU.is_lt)
            c2 = vecp.tile([128, NT], F32, tag="c2")
            nc.vector.tensor_scalar(c2[:], pt[:], 0.2, None, op0=ALU.is_lt)
            gam = vecp.tile([128, NT], F32, tag="gam")
            nc.vector.tensor_add(gam[:], c1[:], c2[:])
            nc.vector.tensor_scalar(gam[:], gam[:], 2.0, 1.0,
                                    op0=ALU.mult, op1=ALU.add)
            u = vecp.tile([128, NT], F32, tag="u")
            nc.vector.tensor_scalar(u[:], pt[:], -1.0, 1.0,
                                    op0=ALU.mult, op1=ALU.add)
            lg = vecp.tile([128, NT], F32, tag="lg")
            nc.scalar.activation(lg[:], u[:], AF.Ln)
            w = vecp.tile([128, NT], F32, tag="w")
            nc.vector.tensor_mul(w[:], gam[:], lg[:])
            nc.scalar.activation(w[:], w[:], AF.Exp)
            foc = vecp.tile([128, NT], F32, tag="foc")
            nc.vector.tensor_mul(foc[:], w[:], ce[:])

            # ---- reduce to partial sums, then across partitions via PE ----
            acc = vecp.tile([128, 8], F32, tag="acc")
            nc.vector.reduce_sum(acc[:, 0:1], kl[:], axis=AX.X)
            nc.vector.reduce_sum(acc[:, 1:2], ce[:], axis=AX.X)
            nc.vector.reduce_sum(acc[:, 2:3], adv[:], axis=AX.X)
            nc.vector.reduce_sum(acc[:, 3:4], foc[:], axis=AX.X)
            nc.vector.reduce_sum(acc[:, 4:5], trace_t[:], axis=AX.X)
            nc.vector.memset(acc[:, 5:8], 0.0)

            ps_f = psp.tile([8, 1], F32, tag="pss")
            nc.tensor.matmul(ps_f[:], acc[:], onesf_t[:],
                             start=True, stop=True)
            out_sb = vecp.tile([8, 1], F32, tag="out_sb")
            nc.scalar.copy(out_sb[:], ps_f[:])
            dma(out=res_d[:], in_=out_sb[:])
            dma(out=rsp_d[:], in_=rsp_t[:])

    nc.compile()
    return nc


_NC = None


def _get_nc():
    global _NC
    if _NC is None:
        _NC = _build_module()
    return _NC


def _prep_inputs(output, target, master_net_pred, feat_pooled,
                 feat_pooled_masked, output_adv, target_adv):
    o = np.asarray(output, dtype=np.float32).astype(ml_dtypes.bfloat16)
    m = np.asarray(master_net_pred,
                   dtype=np.float32).astype(ml_dtypes.bfloat16)
    a = np.asarray(output_adv, dtype=np.float32).astype(ml_dtypes.bfloat16)
    tg = np.asarray(target).astype(np.int64)
    ta = np.asarray(target_adv).astype(np.int64)
    f0 = np.asarray(feat_pooled, dtype=np.float32)
    f1 = np.asarray(feat_pooled_masked, dtype=np.float32)
    feats = np.concatenate([f0, f1], axis=0)  # [2B, D]

    in_maps = []
    for cc in range(NCORES):
        sl = slice(cc * RB, (cc + 1) * RB)
        # GLOBAL roll: preserves mod-8192 circulant distances, so the
        # half-open band covers each unordered pair exactly once fleet-wide.
        order = (np.arange(N2) + cc * RB) % N2
        ftc = np.ascontiguousarray(
            feats[order].T.astype(ml_dtypes.bfloat16))  # [D, 2B]
        in_maps.append({
            "o": np.ascontiguousarray(o[sl]),
            "m": np.ascontiguousarray(m[sl]),
            "a": np.ascontiguousarray(a[sl]),
            "tg": np.ascontiguousarray(
                tg[sl].reshape(NT, 128).T.astype(np.float16)),
            "ta": np.ascontiguousarray(
                ta[sl].reshape(NT, 128).T.astype(np.float16)),
            "ft": ftc,
        })
    return in_maps


def _combine(results):
    r = np.zeros(8, dtype=np.float64)
    rs = np.zeros(N2, dtype=np.float64)
    for cc, rr in enumerate(results):
        r += rr["res"].reshape(-1).astype(np.float64)
        rsp = rr["rsp"].astype(np.float64)        # [128, NQ*5]
        cspv = rr["csp"].astype(np.float64)       # [NQ, ESW]
        rolled = np.zeros(N2, dtype=np.float64)
        for k, Bq in enumerate(QBLKS):
            rows = slice(128 * Bq, 128 * Bq + 128)
            rolled[rows] += rsp[:, 5 * k:5 * k + 5].sum(axis=1)
            cols = (128 * (Bq + 1) + np.arange(BANDC)) % N2
            rolled[cols] += cspv[k, :BANDC]
            rolled[rows] += cspv[k, BANDC:ESW]
        order = (np.arange(N2) + cc * RB) % N2
        rs[order] += rolled
    kl_mean = r[0] / (B * C)
    ce_mean = r[1] / B
    adv_mean = r[2] / B
    foc_mean = r[3] / B
    pos_sum = 2.0 * r[4] / NCE_TEMP          # sum of positive logits, all rows
    lse = np.log(rs)
    nce_mean = (lse.sum() - pos_sum) / N2
    loss = (KL_INTERP * KL_TEMP * KL_TEMP) * kl_mean \
        + (1.0 - KL_INTERP) * ce_mean + nce_mean + foc_mean + adv_mean
    return np.asarray([loss], dtype=np.float32)


def kernel(**inputs):
    in_maps = _prep_inputs(**inputs)
    out = run_bass_kernel_spmd(_get_nc(), in_maps,
                               core_ids=list(range(NCORES)))
    return _combine(out.results)


if __name__ == "__main__":
    rng = np.random.default_rng(0)
    ins = {
        "output": rng.standard_normal((B, C), dtype=np.float32),
        "target": rng.integers(0, C, size=(B,)),
        "master_net_pred": rng.standard_normal((B, C), dtype=np.float32),
        "feat_pooled": rng.standard_normal((B, D), dtype=np.float32),
        "feat_pooled_masked": rng.standard_normal((B, D), dtype=np.float32),
        "output_adv": rng.standard_normal((B, C), dtype=np.float32),
        "target_adv": rng.integers(0, C, size=(B,)),
    }
    print(kernel(**ins))


# revision 16
# speedup vs baseline: 2.0446x; 2.0446x over previous
"""Trainium2 Bass kernel for the combined loss (KL + CE + InfoNCE + focal + adv CE).

Strategy (8 NeuronCores, v2 — symmetric half-band InfoNCE):
  - o / master / o_adv sharded by rows (512/core), shipped bf16.
  - InfoNCE: feats = concat(feat_pooled, feat_pooled_masked) -> [8192, 256],
    transposed to [256, 8192] bf16 with per-core column roll (core's 1024 query
    rows at rolled block positions {0..3, 32..35}).  Exploits G = G^T: each
    query block computes only the half-open circulant band of col-blocks
    (+1..+32), with triangular masks on the self block and the +32 block so
    every unordered pair is computed exactly once.  Each computed exp(l_ij)
    serves row i via the ScalarE accumulate (row sums) and row j via a GpSimd
    partition all-reduce over the exp tile (column sums).  Per-core partial
    row/col sums + positive-pair traces go back to the host, which assembles
    the full 8192 row sums, takes the log, and averages (the only cross-core
    reduction in the algorithm).
  - Normalization: squares on DVE (bf16 2x), column sums-of-squares via a
    bf16 ones-matmul (broadcast across partitions), 1/sqrt on ScalarE Rsqrt,
    normalize multiply on DVE -- keeps the big ACT passes for exp only.
"""

import numpy as np
import ml_dtypes

import concourse.bacc as bacc
import concourse.tile as tile
from concourse import mybir
from concourse.bass_utils import run_bass_kernel_spmd

F32 = mybir.dt.float32
BF16 = mybir.dt.bfloat16
FP16 = mybir.dt.float16
AF = mybir.ActivationFunctionType
ALU = mybir.AluOpType
AX = mybir.AxisListType

NCORES = 8
B, C, D = 4096, 1000, 256
RB = B // NCORES          # 512 rows of the [B, C] tensors per core
NT = RB // 128            # 4 row-tiles per core
N2 = 2 * B                # 8192 infoNCE rows
NBLK = N2 // 128          # 64 column blocks
QBLKS = [0, 1, 2, 3, 32, 33, 34, 35]   # rolled block positions of queries
NQ = len(QBLKS)
BANDC = 32 * 128          # 4096 band columns per query block
ESW = BANDC + 128         # es tile width: band + self block
KCH = 1024                # gram / psum chunk width
SUB = 512                 # matmul moving free dim

KL_TEMP = 4.0
KL_INTERP = 0.5
NCE_TEMP = 0.07
NEG_BIG = -1.0e9


def _col_runs(start, e0, e1):
    """Contiguous rolled-column runs covering band offsets [e0, e1)."""
    runs = []
    e = e0
    while e < e1:
        rs = (start + e) % N2
        ln = min(e1 - e, N2 - rs)
        runs.append((rs, e, ln))
        e += ln
    return runs


def _build_module():
    nc = bacc.Bacc("TRN2", target_bir_lowering=False, debug=False)

    o_d = nc.dram_tensor("o", [RB, C], BF16, kind="ExternalInput")
    m_d = nc.dram_tensor("m", [RB, C], BF16, kind="ExternalInput")
    a_d = nc.dram_tensor("a", [RB, C], BF16, kind="ExternalInput")
    tg_d = nc.dram_tensor("tg", [128, NT], FP16, kind="ExternalInput")
    ta_d = nc.dram_tensor("ta", [128, NT], FP16, kind="ExternalInput")
    ft_d = nc.dram_tensor("ft", [256, N2], BF16, kind="ExternalInput")
    res_d = nc.dram_tensor("res", [8, 1], F32, kind="ExternalOutput")
    rsp_d = nc.dram_tensor("rsp", [128, NQ * 5], F32, kind="ExternalOutput")
    # colsum partials: csp[p, blk] = partial row sum for rolled row 128*blk+p
    csp_d = nc.dram_tensor("csp", [128, NBLK], F32, kind="ExternalOutput")

    iota_np = np.tile(np.arange(C, dtype=np.float16), (128, 1))
    ident_np = np.eye(128, dtype=np.float32)
    identb_np = np.eye(128).astype(ml_dtypes.bfloat16)
    onesb_np = np.ones((128, 128)).astype(ml_dtypes.bfloat16)
    onesf_np = np.ones((128, 1), dtype=np.float32)
    r_idx = np.arange(128)[:, None]
    c_idx = np.arange(128)[None, :]
    # matmul adds lhsT^T @ I = lhsT^T: bake the transpose into the constants.
    # keep c<=r  (kill c>r):  lhsT = NEG*(r>c)
    mask_strict_np = (NEG_BIG * (r_idx > c_idx)).astype(ml_dtypes.bfloat16)
    # keep c<r   (kill c>=r): lhsT = NEG*(r>=c)
    mask_incl_np = (NEG_BIG * (r_idx >= c_idx)).astype(ml_dtypes.bfloat16)

    iota_d = nc.inline_tensor(iota_np, "iota_c")
    ident_d = nc.inline_tensor(ident_np, "ident_c")
    identb_d = nc.inline_tensor(identb_np, "identb_c")
    onesb_d = nc.inline_tensor(onesb_np, "onesb_c")
    onesf_d = nc.inline_tensor(onesf_np, "onesf_c")
    masks_d = nc.inline_tensor(mask_strict_np, "masks_c")
    maski_d = nc.inline_tensor(mask_incl_np, "maski_c")

    from contextlib import ExitStack
    with tile.TileContext(nc) as tc:
        with (
            tc.tile_pool(name="persist", bufs=1) as persist,
            tc.tile_pool(name="io", bufs=2) as iop,
            tc.tile_pool(name="scr", bufs=4) as scrp,
            tc.tile_pool(name="vec", bufs=1) as vecp,
            tc.tile_pool(name="ps", bufs=2, space="PSUM") as psp,
            ExitStack() as late_ctx,
        ):
            dma = nc.default_dma_engine.dma_start

            iota_t = persist.tile([128, C], FP16, tag="iota")
            dma(out=iota_t[:], in_=iota_d[:])
            ident_t = persist.tile([128, 128], F32, tag="ident")
            dma(out=ident_t[:], in_=ident_d[:])
            identb_t = persist.tile([128, 128], BF16, tag="identb")
            dma(out=identb_t[:], in_=identb_d[:])
            onesb_t = persist.tile([128, 128], BF16, tag="onesb")
            dma(out=onesb_t[:], in_=onesb_d[:])
            onesf_t = persist.tile([128, 1], F32, tag="onesf")
            dma(out=onesf_t[:], in_=onesf_d[:])
            masks_t = persist.tile([128, 128], BF16, tag="masks")
            dma(out=masks_t[:], in_=masks_d[:])
            maski_t = persist.tile([128, 128], BF16, tag="maski")
            dma(out=maski_t[:], in_=maski_d[:])
            tg_t = persist.tile([128, NT], FP16, tag="tg")
            dma(out=tg_t[:], in_=tg_d[:])
            ta_t = persist.tile([128, NT], FP16, tag="ta")
            dma(out=ta_t[:], in_=ta_d[:])

            hn0 = persist.tile([128, N2], BF16, tag="hn0")
            hn1 = persist.tile([128, N2], BF16, tag="hn1")
            essum = persist.tile([128, N2], BF16, tag="essum")
            rsp_t = persist.tile([128, NQ * 5], F32, tag="rsp")
            trace_t = vecp.tile([128, 4], F32, tag="trace")
            nc.vector.memset(essum[:], 0.0)

            # cekl per-row stat slots
            S1 = vecp.tile([128, NT], F32, tag="S1")
            ST = vecp.tile([128, NT], F32, tag="ST")
            SM = vecp.tile([128, NT], F32, tag="SM")
            SA = vecp.tile([128, NT], F32, tag="SA")
            PP = vecp.tile([128, NT], F32, tag="PP")
            GO = vecp.tile([128, NT], F32, tag="GO")
            GA = vecp.tile([128, NT], F32, tag="GA")

            def cekl_tile(t):
                rsl = slice(t * 128, (t + 1) * 128)
                o_t = iop.tile([128, C], BF16, tag="o")
                dma(out=o_t[:], in_=o_d[rsl, :])
                m_t = iop.tile([128, C], BF16, tag="m")
                dma(out=m_t[:], in_=m_d[rsl, :])
                a_t = iop.tile([128, C], BF16, tag="a")
                dma(out=a_t[:], in_=a_d[rsl, :])

                e1 = scrp.tile([128, C], BF16, tag="scr1000")
                nc.scalar.activation(e1[:], o_t[:], AF.Exp, scale=1.0,
                                     accum_out=S1[:, t:t + 1])
                e2 = scrp.tile([128, C], BF16, tag="scr1000")
                nc.scalar.activation(e2[:], o_t[:], AF.Exp,
                                     scale=float(1.0 / KL_TEMP),
                                     accum_out=ST[:, t:t + 1])
                em_t = iop.tile([128, C], BF16, tag="em")
                nc.scalar.activation(em_t[:], m_t[:], AF.Exp,
                                     scale=float(1.0 / KL_TEMP),
                                     accum_out=SM[:, t:t + 1])
                e3 = scrp.tile([128, C], BF16, tag="scr1000")
                nc.scalar.activation(e3[:], a_t[:], AF.Exp, scale=1.0,
                                     accum_out=SA[:, t:t + 1])

                d_t = iop.tile([128, C], BF16, tag="d")
                nc.vector.tensor_sub(d_t[:], m_t[:], o_t[:])
                pr = scrp.tile([128, C], BF16, tag="scr1000")
                nc.vector.scalar_tensor_tensor(
                    out=pr[:], in0=d_t[:], scalar=1.0, in1=em_t[:],
                    op0=ALU.mult, op1=ALU.mult, accum_out=PP[:, t:t + 1])
                g1 = scrp.tile([128, C], BF16, tag="scr1000")
                nc.vector.scalar_tensor_tensor(
                    out=g1[:], in0=iota_t[:], scalar=tg_t[:, t:t + 1],
                    in1=o_t[:], op0=ALU.is_equal, op1=ALU.mult,
                    accum_out=GO[:, t:t + 1])
                g2 = scrp.tile([128, C], BF16, tag="scr1000")
                nc.vector.scalar_tensor_tensor(
                    out=g2[:], in0=iota_t[:], scalar=ta_t[:, t:t + 1],
                    in1=a_t[:], op0=ALU.is_equal, op1=ALU.mult,
                    accum_out=GA[:, t:t + 1])

            # ---- prologue: cekl tile 0 keeps ACT busy during feature DMA ----
            cekl_tile(0)

            raw_pool_cm = tc.tile_pool(name="raw", bufs=1)
            rawp = raw_pool_cm.__enter__()
            h0 = rawp.tile([128, N2], BF16, tag="h0")
            h1 = rawp.tile([128, N2], BF16, tag="h1")
            sq0 = rawp.tile([128, N2], BF16, tag="sq0")
            sq1 = rawp.tile([128, N2], BF16, tag="sq1")
            rn = rawp.tile([128, N2], BF16, tag="rn")
            NCH = N2 // KCH
            for ch in range(NCH):
                sl = slice(ch * KCH, (ch + 1) * KCH)
                dma(out=h0[:, sl], in_=ft_d[0:128, sl])
                dma(out=h1[:, sl], in_=ft_d[128:256, sl])

            cekl_tile(1)

            # ---- normalization pipeline, chunked ----
            # rn = s^-0.5 via exp(-0.5*ln(s)); all Lns grouped, then one Exp,
            # to avoid activation-table reloads (Rsqrt is blocked by bass).
            for ch in range(NCH):
                sl = slice(ch * KCH, (ch + 1) * KCH)
                nc.vector.tensor_mul(sq0[:, sl], h0[:, sl], h0[:, sl])
                nc.vector.tensor_mul(sq1[:, sl], h1[:, sl], h1[:, sl])
                ps_n = psp.tile([128, KCH], F32, tag="ps")
                for half, sq in ((0, sq0), (1, sq1)):
                    for s in range(KCH // SUB):
                        c0 = ch * KCH + s * SUB
                        osl = slice(s * SUB, (s + 1) * SUB)
                        nc.tensor.matmul(ps_n[:, osl], onesb_t[:],
                                         sq[:, c0:c0 + SUB],
                                         start=(half == 0), stop=(half == 1))
                nc.scalar.activation(rn[:, sl], ps_n[:], AF.Ln)
            nc.scalar.activation(rn[:], rn[:], AF.Exp, scale=-0.5)
            for ch in range(NCH):
                sl = slice(ch * KCH, (ch + 1) * KCH)
                nc.vector.tensor_mul(hn0[:, sl], h0[:, sl], rn[:, sl])
                nc.vector.tensor_mul(hn1[:, sl], h1[:, sl], rn[:, sl])

            cekl_tile(2)
            cekl_tile(3)

            # raw features / squares / rn are dead; release before es/cs pools.
            raw_pool_cm.__exit__(None, None, None)
            esp = late_ctx.enter_context(tc.tile_pool(name="es", bufs=3))

            # ---- symmetric half-band InfoNCE ----
            for qi, Bq in enumerate(QBLKS):
                cb = 128 * Bq
                start = 128 * (Bq + 1)
                lhs0 = hn0[:, cb:cb + 128]
                lhs1 = hn1[:, cb:cb + 128]
                es_t = esp.tile([128, ESW], BF16, tag="es")
                for kc in range(4):
                    ps_k = psp.tile([128, KCH], F32, tag="ps")
                    for half, hn, lhsT in ((0, hn0, lhs0), (1, hn1, lhs1)):
                        for s in range(KCH // SUB):
                            base = kc * KCH + s * SUB
                            for (rs, e, ln) in _col_runs(start, base,
                                                         base + SUB):
                                off = e - kc * KCH
                                nc.tensor.matmul(
                                    ps_k[:, off:off + ln], lhsT,
                                    hn[:, rs:rs + ln],
                                    start=(half == 0), stop=(half == 1))
                        if half == 0 and kc == 3:
                            # +32 block triangular mask (last 128 cols)
                            mk = masks_t if qi < 4 else maski_t
                            nc.tensor.matmul(ps_k[:, 896:1024], mk[:],
                                             identb_t[:], start=False,
                                             stop=False,
                                             skip_group_check=True)
                    nc.scalar.activation(
                        es_t[:, kc * KCH:(kc + 1) * KCH], ps_k[:], AF.Exp,
                        scale=float(1.0 / NCE_TEMP),
                        accum_out=rsp_t[:, qi * 5 + kc:qi * 5 + kc + 1])
                    if kc == 3 and qi < 4:
                        tr_scr = scrp.tile([128, 128], F32, tag="trscr")
                        nc.vector.scalar_tensor_tensor(
                            out=tr_scr[:], in0=ps_k[:, 896:1024], scalar=1.0,
                            in1=ident_t[:], op0=ALU.mult, op1=ALU.mult,
                            accum_out=trace_t[:, qi:qi + 1])
                # self block (strict lower kept)
                ps_s = psp.tile([128, 128], F32, tag="pss")
                nc.tensor.matmul(ps_s[:], lhs0, hn0[:, cb:cb + 128],
                                 start=True, stop=False)
                nc.tensor.matmul(ps_s[:], lhs1, hn1[:, cb:cb + 128],
                                 start=False, stop=False)
                nc.tensor.matmul(ps_s[:], maski_t[:], identb_t[:],
                                 start=False, stop=True)
                nc.scalar.activation(
                    es_t[:, BANDC:ESW], ps_s[:], AF.Exp,
                    scale=float(1.0 / NCE_TEMP),
                    accum_out=rsp_t[:, qi * 5 + 4:qi * 5 + 5])
                # accumulate exp tile into the rolled-column es accumulator
                # (column sums ignore row identity, so summing the 8 query
                # blocks' tiles first lets one matmul pass extract all 64
                # block column sums at the end)
                for (rs_c, e, ln) in _col_runs(start, 0, BANDC):
                    nc.vector.tensor_add(essum[:, rs_c:rs_c + ln],
                                         essum[:, rs_c:rs_c + ln],
                                         es_t[:, e:e + ln])
                nc.vector.tensor_add(essum[:, cb:cb + 128],
                                     essum[:, cb:cb + 128],
                                     es_t[:, BANDC:ESW])

            # ---- extract all 64 block column sums: lhsT = essum block,
            # rhs = ones column -> ct[p, blk] = colsum(rolled col 128*blk+p)
            ct_ps = psp.tile([128, NBLK], F32, tag="ct")
            for bk in range(NBLK):
                nc.tensor.matmul(ct_ps[:, bk:bk + 1],
                                 essum[:, bk * 128:(bk + 1) * 128],
                                 onesb_t[:, 0:1], start=True, stop=True)
            ct_sb = vecp.tile([128, NBLK], F32, tag="ct_sb")
            nc.scalar.copy(ct_sb[:], ct_ps[:])
            dma(out=csp_d[:], in_=ct_sb[:])

            # ---- epilogue on [128, NT] stat vectors ----
            lse1 = vecp.tile([128, NT], F32, tag="lse1")
            nc.scalar.activation(lse1[:], S1[:], AF.Ln)
            lseT = vecp.tile([128, NT], F32, tag="lseT")
            nc.scalar.activation(lseT[:], ST[:], AF.Ln)
            lsem = vecp.tile([128, NT], F32, tag="lsem")
            nc.scalar.activation(lsem[:], SM[:], AF.Ln)
            lsea = vecp.tile([128, NT], F32, tag="lsea")
            nc.scalar.activation(lsea[:], SA[:], AF.Ln)

            ce = vecp.tile([128, NT], F32, tag="ce")
            nc.vector.tensor_sub(ce[:], lse1[:], GO[:])
            adv = vecp.tile([128, NT], F32, tag="adv")
            nc.vector.tensor_sub(adv[:], lsea[:], GA[:])

            # kl_row = PP/(T*SM) - lsem + lseT
            invSM = vecp.tile([128, NT], F32, tag="invSM")
            nc.vector.reciprocal(invSM[:], SM[:])
            kl = vecp.tile([128, NT], F32, tag="kl")
            nc.vector.tensor_mul(kl[:], PP[:], invSM[:])
            nc.vector.tensor_scalar_mul(kl[:], kl[:], float(1.0 / KL_TEMP))
            nc.vector.tensor_sub(kl[:], kl[:], lsem[:])
            nc.vector.tensor_add(kl[:], kl[:], lseT[:])

            # focal_row = (1-pt)^gamma * ce,  pt = exp(-ce)
            pt = vecp.tile([128, NT], F32, tag="pt")
            nc.scalar.activation(pt[:], ce[:], AF.Exp, scale=-1.0)
            c1 = vecp.tile([128, NT], F32, tag="c1")
            nc.vector.tensor_scalar(c1[:], pt[:], 0.5, None, op0=ALU.is_lt)
            c2 = vecp.tile([128, NT], F32, tag="c2")
            nc.vector.tensor_scalar(c2[:], pt[:], 0.2, None, op0=ALU.is_lt)
            # w = (1-pt)^gamma with gamma = 1 + 2*c1 + 2*c2:
            #   w = u * (u^2)^c1 * (u^2)^c2,  (u^2)^ci = 1 + ci*(u^2-1)
            u = vecp.tile([128, NT], F32, tag="u")
            nc.vector.tensor_scalar(u[:], pt[:], -1.0, 1.0,
                                    op0=ALU.mult, op1=ALU.add)
            t2 = vecp.tile([128, NT], F32, tag="t2")
            nc.vector.tensor_mul(t2[:], u[:], u[:])
            nc.vector.tensor_scalar(t2[:], t2[:], -1.0, None, op0=ALU.add)
            f1 = vecp.tile([128, NT], F32, tag="f1")
            nc.vector.tensor_mul(f1[:], c1[:], t2[:])
            nc.vector.tensor_scalar(f1[:], f1[:], 1.0, None, op0=ALU.add)
            f2 = vecp.tile([128, NT], F32, tag="f2")
            nc.vector.tensor_mul(f2[:], c2[:], t2[:])
            nc.vector.tensor_scalar(f2[:], f2[:], 1.0, None, op0=ALU.add)
            w = vecp.tile([128, NT], F32, tag="w")
            nc.vector.tensor_mul(w[:], u[:], f1[:])
            nc.vector.tensor_mul(w[:], w[:], f2[:])
            foc = vecp.tile([128, NT], F32, tag="foc")
            nc.vector.tensor_mul(foc[:], w[:], ce[:])

            # ---- reduce to partial sums, then across partitions via PE ----
            acc = vecp.tile([128, 8], F32, tag="acc")
            nc.vector.reduce_sum(acc[:, 0:1], kl[:], axis=AX.X)
            nc.vector.reduce_sum(acc[:, 1:2], ce[:], axis=AX.X)
            nc.vector.reduce_sum(acc[:, 2:3], adv[:], axis=AX.X)
            nc.vector.reduce_sum(acc[:, 3:4], foc[:], axis=AX.X)
            nc.vector.reduce_sum(acc[:, 4:5], trace_t[:], axis=AX.X)
            nc.vector.memset(acc[:, 5:8], 0.0)

            ps_f = psp.tile([8, 1], F32, tag="pss")
            nc.tensor.matmul(ps_f[:], acc[:], onesf_t[:],
                             start=True, stop=True)
            out_sb = vecp.tile([8, 1], F32, tag="out_sb")
            nc.scalar.copy(out_sb[:], ps_f[:])
            dma(out=res_d[:], in_=out_sb[:])
            dma(out=rsp_d[:], in_=rsp_t[:])

    nc.compile()
    return nc


_NC = None


def _get_nc():
    global _NC
    if _NC is None:
        _NC = _build_module()
    return _NC


def _prep_inputs(output, target, master_net_pred, feat_pooled,
                 feat_pooled_masked, output_adv, target_adv):
    o = np.asarray(output, dtype=np.float32).astype(ml_dtypes.bfloat16)
    m = np.asarray(master_net_pred,
                   dtype=np.float32).astype(ml_dtypes.bfloat16)
    a = np.asarray(output_adv, dtype=np.float32).astype(ml_dtypes.bfloat16)
    tg = np.asarray(target).astype(np.int64)
    ta = np.asarray(target_adv).astype(np.int64)
    f0 = np.asarray(feat_pooled, dtype=np.float32)
    f1 = np.asarray(feat_pooled_masked, dtype=np.float32)
    feats = np.concatenate([f0, f1], axis=0)  # [2B, D]

    in_maps = []
    for cc in range(NCORES):
        sl = slice(cc * RB, (cc + 1) * RB)
        # GLOBAL roll: preserves mod-8192 circulant distances, so the
        # half-open band covers each unordered pair exactly once fleet-wide.
        order = (np.arange(N2) + cc * RB) % N2
        ftc = np.ascontiguousarray(
            feats[order].T.astype(ml_dtypes.bfloat16))  # [D, 2B]
        in_maps.append({
            "o": np.ascontiguousarray(o[sl]),
            "m": np.ascontiguousarray(m[sl]),
            "a": np.ascontiguousarray(a[sl]),
            "tg": np.ascontiguousarray(
                tg[sl].reshape(NT, 128).T.astype(np.float16)),
            "ta": np.ascontiguousarray(
                ta[sl].reshape(NT, 128).T.astype(np.float16)),
            "ft": ftc,
        })
    return in_maps


def _combine(results):
    r = np.zeros(8, dtype=np.float64)
    rs = np.zeros(N2, dtype=np.float64)
    for cc, rr in enumerate(results):
        r += rr["res"].reshape(-1).astype(np.float64)
        rsp = rr["rsp"].astype(np.float64)        # [128, NQ*5]
        cspv = rr["csp"].astype(np.float64)       # [128, NBLK]
        rolled = cspv.T.reshape(-1).copy()        # rolled col 128*blk+p
        for k, Bq in enumerate(QBLKS):
            rows = slice(128 * Bq, 128 * Bq + 128)
            rolled[rows] += rsp[:, 5 * k:5 * k + 5].sum(axis=1)
        order = (np.arange(N2) + cc * RB) % N2
        rs[order] += rolled
    kl_mean = r[0] / (B * C)
    ce_mean = r[1] / B
    adv_mean = r[2] / B
    foc_mean = r[3] / B
    pos_sum = 2.0 * r[4] / NCE_TEMP          # sum of positive logits, all rows
    lse = np.log(rs)
    nce_mean = (lse.sum() - pos_sum) / N2
    loss = (KL_INTERP * KL_TEMP * KL_TEMP) * kl_mean \
        + (1.0 - KL_INTERP) * ce_mean + nce_mean + foc_mean + adv_mean
    return np.asarray([loss], dtype=np.float32)


def kernel(**inputs):
    in_maps = _prep_inputs(**inputs)
    out = run_bass_kernel_spmd(_get_nc(), in_maps,
                               core_ids=list(range(NCORES)))
    return _combine(out.results)


if __name__ == "__main__":
    rng = np.random.default_rng(0)
    ins = {
        "output": rng.standard_normal((B, C), dtype=np.float32),
        "target": rng.integers(0, C, size=(B,)),
        "master_net_pred": rng.standard_normal((B, C), dtype=np.float32),
        "feat_pooled": rng.standard_normal((B, D), dtype=np.float32),
        "feat_pooled_masked": rng.standard_normal((B, D), dtype=np.float32),
        "output_adv": rng.standard_normal((B, C), dtype=np.float32),
        "target_adv": rng.integers(0, C, size=(B,)),
    }
    print(kernel(**ins))


# revision 25
# speedup vs baseline: 2.0775x; 1.0161x over previous
"""Trainium2 Bass kernel for the combined loss (KL + CE + InfoNCE + focal + adv CE).

Strategy (8 NeuronCores, v2 — symmetric half-band InfoNCE):
  - o / master / o_adv sharded by rows (512/core), shipped bf16.
  - InfoNCE: feats = concat(feat_pooled, feat_pooled_masked) -> [8192, 256],
    transposed to [256, 8192] bf16 with per-core column roll (core's 1024 query
    rows at rolled block positions {0..3, 32..35}).  Exploits G = G^T: each
    query block computes only the half-open circulant band of col-blocks
    (+1..+32), with triangular masks on the self block and the +32 block so
    every unordered pair is computed exactly once.  Each computed exp(l_ij)
    serves row i via the ScalarE accumulate (row sums) and row j via a GpSimd
    partition all-reduce over the exp tile (column sums).  Per-core partial
    row/col sums + positive-pair traces go back to the host, which assembles
    the full 8192 row sums, takes the log, and averages (the only cross-core
    reduction in the algorithm).
  - Normalization: squares on DVE (bf16 2x), column sums-of-squares via a
    bf16 ones-matmul (broadcast across partitions), 1/sqrt on ScalarE Rsqrt,
    normalize multiply on DVE -- keeps the big ACT passes for exp only.
"""

import numpy as np
import ml_dtypes

import concourse.bacc as bacc
import concourse.tile as tile
from concourse import mybir
from concourse.bass_utils import run_bass_kernel_spmd

F32 = mybir.dt.float32
BF16 = mybir.dt.bfloat16
FP16 = mybir.dt.float16
AF = mybir.ActivationFunctionType
ALU = mybir.AluOpType
AX = mybir.AxisListType

NCORES = 8
B, C, D = 4096, 1000, 256
RB = B // NCORES          # 512 rows of the [B, C] tensors per core
NT = RB // 128            # 4 row-tiles per core
N2 = 2 * B                # 8192 infoNCE rows
NBLK = N2 // 128          # 64 column blocks
QBLKS = [0, 1, 2, 3, 32, 33, 34, 35]   # rolled block positions of queries
NQ = len(QBLKS)
BANDC = 32 * 128          # 4096 band columns per query block
ESW = BANDC + 128         # es tile width: band + self block
KCH = 1024                # gram / psum chunk width
SUB = 512                 # matmul moving free dim

KL_TEMP = 4.0
KL_INTERP = 0.5
NCE_TEMP = 0.07
NEG_BIG = -1.0e9


def _col_runs(start, e0, e1):
    """Contiguous rolled-column runs covering band offsets [e0, e1)."""
    runs = []
    e = e0
    while e < e1:
        rs = (start + e) % N2
        ln = min(e1 - e, N2 - rs)
        runs.append((rs, e, ln))
        e += ln
    return runs


def _build_module():
    nc = bacc.Bacc("TRN2", target_bir_lowering=False, debug=False)

    o_d = nc.dram_tensor("o", [RB, C], BF16, kind="ExternalInput")
    m_d = nc.dram_tensor("m", [RB, C], BF16, kind="ExternalInput")
    a_d = nc.dram_tensor("a", [RB, C], BF16, kind="ExternalInput")
    tg_d = nc.dram_tensor("tg", [128, NT], FP16, kind="ExternalInput")
    ta_d = nc.dram_tensor("ta", [128, NT], FP16, kind="ExternalInput")
    ft_d = nc.dram_tensor("ft", [256, N2], BF16, kind="ExternalInput")
    res_d = nc.dram_tensor("res", [8, 1], F32, kind="ExternalOutput")
    rsp_d = nc.dram_tensor("rsp", [128, NQ * 3], F32, kind="ExternalOutput")
    # colsum partials: csp[p, blk] = partial row sum for rolled row 128*blk+p
    csp_d = nc.dram_tensor("csp", [128, NBLK], F32, kind="ExternalOutput")

    iota_np = np.tile(np.arange(C, dtype=np.float16), (128, 1))
    ident_np = np.eye(128, dtype=np.float32)
    identb_np = np.eye(128).astype(ml_dtypes.bfloat16)
    onesb_np = np.ones((128, 128)).astype(ml_dtypes.bfloat16)
    onesf_np = np.ones((128, 1), dtype=np.float32)
    r_idx = np.arange(128)[:, None]
    c_idx = np.arange(128)[None, :]
    # matmul adds lhsT^T @ I = lhsT^T: bake the transpose into the constants.
    # keep c<=r  (kill c>r):  lhsT = NEG*(r>c)
    mask_strict_np = (NEG_BIG * (r_idx > c_idx)).astype(ml_dtypes.bfloat16)
    # keep c<r   (kill c>=r): lhsT = NEG*(r>=c)
    mask_incl_np = (NEG_BIG * (r_idx >= c_idx)).astype(ml_dtypes.bfloat16)

    iota_d = nc.inline_tensor(iota_np, "iota_c")
    ident_d = nc.inline_tensor(ident_np, "ident_c")
    identb_d = nc.inline_tensor(identb_np, "identb_c")
    onesb_d = nc.inline_tensor(onesb_np, "onesb_c")
    onesf_d = nc.inline_tensor(onesf_np, "onesf_c")
    masks_d = nc.inline_tensor(mask_strict_np, "masks_c")
    maski_d = nc.inline_tensor(mask_incl_np, "maski_c")

    from contextlib import ExitStack
    with tile.TileContext(nc) as tc:
        with (
            tc.tile_pool(name="persist", bufs=1) as persist,
            tc.tile_pool(name="io", bufs=2) as iop,
            tc.tile_pool(name="scr", bufs=4) as scrp,
            tc.tile_pool(name="vec", bufs=1) as vecp,
            tc.tile_pool(name="ps", bufs=2, space="PSUM") as psp,
            ExitStack() as late_ctx,
        ):
            dma = nc.default_dma_engine.dma_start

            iota_t = persist.tile([128, C], FP16, tag="iota")
            dma(out=iota_t[:], in_=iota_d[:])
            ident_t = persist.tile([128, 128], F32, tag="ident")
            dma(out=ident_t[:], in_=ident_d[:])
            identb_t = persist.tile([128, 128], BF16, tag="identb")
            dma(out=identb_t[:], in_=identb_d[:])
            onesb_t = persist.tile([128, 128], BF16, tag="onesb")
            dma(out=onesb_t[:], in_=onesb_d[:])
            onesf_t = persist.tile([128, 1], F32, tag="onesf")
            dma(out=onesf_t[:], in_=onesf_d[:])
            masks_t = persist.tile([128, 128], BF16, tag="masks")
            dma(out=masks_t[:], in_=masks_d[:])
            maski_t = persist.tile([128, 128], BF16, tag="maski")
            dma(out=maski_t[:], in_=maski_d[:])
            tg_t = persist.tile([128, NT], FP16, tag="tg")
            dma(out=tg_t[:], in_=tg_d[:])
            ta_t = persist.tile([128, NT], FP16, tag="ta")
            dma(out=ta_t[:], in_=ta_d[:])

            hn0 = persist.tile([128, N2], BF16, tag="hn0")
            hn1 = persist.tile([128, N2], BF16, tag="hn1")
            essum = persist.tile([128, N2], BF16, tag="essum")
            rsp_t = persist.tile([128, NQ * 3], F32, tag="rsp")
            trace_t = vecp.tile([128, 4], F32, tag="trace")
            nc.vector.memset(essum[:], 0.0)

            # cekl per-row stat slots; the four lse stats share one tile so a
            # single Ln covers them (fewer act-table switches)
            stats = vecp.tile([128, 16], F32, tag="stats")
            PP = vecp.tile([128, NT], F32, tag="PP")
            GO = vecp.tile([128, NT], F32, tag="GO")
            GA = vecp.tile([128, NT], F32, tag="GA")

            def cekl_tile(t):
                rsl = slice(t * 128, (t + 1) * 128)
                o_t = iop.tile([128, C], BF16, tag="o")
                dma(out=o_t[:], in_=o_d[rsl, :])
                m_t = iop.tile([128, C], BF16, tag="m")
                dma(out=m_t[:], in_=m_d[rsl, :])
                a_t = iop.tile([128, C], BF16, tag="a")
                dma(out=a_t[:], in_=a_d[rsl, :])

                e1 = scrp.tile([128, C], BF16, tag="scr1000")
                nc.scalar.activation(e1[:], o_t[:], AF.Exp, scale=1.0,
                                     accum_out=stats[:, t:t + 1])
                e2 = scrp.tile([128, C], BF16, tag="scr1000")
                nc.scalar.activation(e2[:], o_t[:], AF.Exp,
                                     scale=float(1.0 / KL_TEMP),
                                     accum_out=stats[:, 4 + t:5 + t])
                em_t = iop.tile([128, C], BF16, tag="em")
                nc.scalar.activation(em_t[:], m_t[:], AF.Exp,
                                     scale=float(1.0 / KL_TEMP),
                                     accum_out=stats[:, 8 + t:9 + t])
                e3 = scrp.tile([128, C], BF16, tag="scr1000")
                nc.scalar.activation(e3[:], a_t[:], AF.Exp, scale=1.0,
                                     accum_out=stats[:, 12 + t:13 + t])

                d_t = iop.tile([128, C], BF16, tag="d")
                nc.vector.tensor_sub(d_t[:], m_t[:], o_t[:])
                pr = scrp.tile([128, C], BF16, tag="scr1000")
                nc.vector.scalar_tensor_tensor(
                    out=pr[:], in0=d_t[:], scalar=1.0, in1=em_t[:],
                    op0=ALU.mult, op1=ALU.mult, accum_out=PP[:, t:t + 1])
                g1 = scrp.tile([128, C], BF16, tag="scr1000")
                nc.vector.scalar_tensor_tensor(
                    out=g1[:], in0=iota_t[:], scalar=tg_t[:, t:t + 1],
                    in1=o_t[:], op0=ALU.is_equal, op1=ALU.mult,
                    accum_out=GO[:, t:t + 1])
                g2 = scrp.tile([128, C], BF16, tag="scr1000")
                nc.vector.scalar_tensor_tensor(
                    out=g2[:], in0=iota_t[:], scalar=ta_t[:, t:t + 1],
                    in1=a_t[:], op0=ALU.is_equal, op1=ALU.mult,
                    accum_out=GA[:, t:t + 1])

            # ---- prologue: cekl tiles 0/1 keep ACT busy during feature DMA
            # (their DMAs queue ahead of the features so their exps finish
            # before the norm Lns start -- avoids act-table ping-pong) ----
            cekl_tile(0)
            cekl_tile(1)

            raw_pool_cm = tc.tile_pool(name="raw", bufs=1)
            rawp = raw_pool_cm.__enter__()
            h0 = rawp.tile([128, N2], BF16, tag="h0")
            h1 = rawp.tile([128, N2], BF16, tag="h1")
            sq0 = rawp.tile([128, N2], BF16, tag="sq0")
            sq1 = rawp.tile([128, N2], BF16, tag="sq1")
            rn = rawp.tile([128, N2], BF16, tag="rn")
            NCH = N2 // KCH
            for ch in range(NCH):
                sl = slice(ch * KCH, (ch + 1) * KCH)
                dma(out=h0[:, sl], in_=ft_d[0:128, sl])
                dma(out=h1[:, sl], in_=ft_d[128:256, sl])

            # ---- normalization pipeline, chunked ----
            # rn = s^-0.5 via exp(-0.5*ln(s)); all Lns grouped, then one Exp,
            # to avoid activation-table reloads (Rsqrt is blocked by bass).
            for ch in range(NCH):
                sl = slice(ch * KCH, (ch + 1) * KCH)
                nc.vector.tensor_mul(sq0[:, sl], h0[:, sl], h0[:, sl])
                nc.vector.tensor_mul(sq1[:, sl], h1[:, sl], h1[:, sl])
                ps_n = psp.tile([128, KCH], F32, tag="ps")
                for half, sq in ((0, sq0), (1, sq1)):
                    for s in range(KCH // SUB):
                        c0 = ch * KCH + s * SUB
                        osl = slice(s * SUB, (s + 1) * SUB)
                        nc.tensor.matmul(ps_n[:, osl], onesb_t[:],
                                         sq[:, c0:c0 + SUB],
                                         start=(half == 0), stop=(half == 1))
                nc.scalar.activation(rn[:, sl], ps_n[:], AF.Ln)
            nc.scalar.activation(rn[:], rn[:], AF.Exp, scale=-0.5)
            for ch in range(NCH):
                sl = slice(ch * KCH, (ch + 1) * KCH)
                nc.vector.tensor_mul(hn0[:, sl], h0[:, sl], rn[:, sl])
                nc.vector.tensor_mul(hn1[:, sl], h1[:, sl], rn[:, sl])

            cekl_tile(2)
            cekl_tile(3)

            # raw features / squares / rn are dead; release before es/cs pools.
            raw_pool_cm.__exit__(None, None, None)
            esp = late_ctx.enter_context(tc.tile_pool(name="es", bufs=3))

            # ---- symmetric half-band InfoNCE ----
            # Per query block: self block + 32 band blocks are CONTIGUOUS in
            # rolled space -> one 4224-col extent, chunked [2048, 2048, 128].
            CHUNKS = [(0, 2048), (2048, 2048), (4096, 128)]
            # ct wave 1: blocks untouched by the second-half query extents
            wave1 = [b for b in range(NBLK)
                     if not any((b - Bq) % NBLK <= 32 for Bq in QBLKS[4:])]

            def ct_extract(blocks):
                # shares the "ps" psum slots with the gram chunks
                ct_ps = psp.tile([128, len(blocks)], F32, tag="ps")
                for i, bk in enumerate(blocks):
                    nc.tensor.matmul(ct_ps[:, i:i + 1],
                                     essum[:, bk * 128:(bk + 1) * 128],
                                     onesb_t[:, 0:1], start=True, stop=True)
                ct_sb = vecp.tile([128, len(blocks)], F32,
                                  tag=f"ct_sb{blocks[0]}")
                nc.scalar.copy(ct_sb[:], ct_ps[:])
                # DMA contiguous block runs in one shot each
                i = 0
                while i < len(blocks):
                    j = i
                    while j + 1 < len(blocks) and blocks[j + 1] == blocks[j] + 1:
                        j += 1
                    dma(out=csp_d[:, blocks[i]:blocks[j] + 1],
                        in_=ct_sb[:, i:j + 1])
                    i = j + 1

            for qi, Bq in enumerate(QBLKS):
                cb = 128 * Bq
                lhs0 = hn0[:, cb:cb + 128]
                lhs1 = hn1[:, cb:cb + 128]
                es_t = esp.tile([128, ESW], BF16, tag="es")
                for ci, (base0, cw) in enumerate(CHUNKS):
                    ps_k = psp.tile([128, cw], F32, tag="ps")
                    for half, hn, lhsT in ((0, hn0, lhs0), (1, hn1, lhs1)):
                        for s in range((cw + SUB - 1) // SUB):
                            b0 = base0 + s * SUB
                            b1 = min(base0 + cw, b0 + SUB)
                            for (rs_c, e, ln) in _col_runs(cb, b0, b1):
                                off = e - base0
                                nc.tensor.matmul(
                                    ps_k[:, off:off + ln], lhsT,
                                    hn[:, rs_c:rs_c + ln],
                                    start=(half == 0), stop=(half == 1))
                        if half == 0 and ci == 0:
                            # self block: keep strict lower triangle
                            nc.tensor.matmul(ps_k[:, 0:128], maski_t[:],
                                             identb_t[:], start=False,
                                             stop=False,
                                             skip_group_check=True)
                        if half == 0 and ci == 2:
                            # +32 block: tie-break mask on the pair diagonal
                            mk = masks_t if qi < 4 else maski_t
                            nc.tensor.matmul(ps_k[:, 0:128], mk[:],
                                             identb_t[:], start=False,
                                             stop=False,
                                             skip_group_check=True)
                    nc.scalar.activation(
                        es_t[:, base0:base0 + cw], ps_k[:], AF.Exp,
                        scale=float(1.0 / NCE_TEMP),
                        accum_out=rsp_t[:, qi * 3 + ci:qi * 3 + ci + 1])
                    if ci == 2 and qi < 4:
                        tr_scr = scrp.tile([128, 128], F32, tag="trscr")
                        nc.vector.scalar_tensor_tensor(
                            out=tr_scr[:], in0=ps_k[:, 0:128], scalar=1.0,
                            in1=ident_t[:], op0=ALU.mult, op1=ALU.mult,
                            accum_out=trace_t[:, qi:qi + 1])
                # accumulate exp tile into the rolled-column es accumulator
                # (column sums ignore row identity, so summing the 8 query
                # blocks' tiles first lets one matmul pass extract all 64
                # block column sums)
                for (rs_c, e, ln) in _col_runs(cb, 0, ESW):
                    nc.vector.tensor_add(essum[:, rs_c:rs_c + ln],
                                         essum[:, rs_c:rs_c + ln],
                                         es_t[:, e:e + ln])
                if qi == 3:
                    ct_extract(wave1)

            ct_extract([b for b in range(NBLK) if b not in wave1])

            # ---- epilogue on [128, NT] stat vectors ----
            lse_all = vecp.tile([128, 16], F32, tag="lse_all")
            nc.scalar.activation(lse_all[:], stats[:], AF.Ln)
            lse1 = lse_all[:, 0:4]
            lseT = lse_all[:, 4:8]
            lsem = lse_all[:, 8:12]
            lsea = lse_all[:, 12:16]

            ce = vecp.tile([128, NT], F32, tag="ce")
            nc.vector.tensor_sub(ce[:], lse1, GO[:])
            adv = vecp.tile([128, NT], F32, tag="adv")
            nc.vector.tensor_sub(adv[:], lsea, GA[:])

            # kl_row = PP/(T*SM) - lsem + lseT
            invSM = vecp.tile([128, NT], F32, tag="invSM")
            nc.vector.reciprocal(invSM[:], stats[:, 8:12])
            kl = vecp.tile([128, NT], F32, tag="kl")
            nc.vector.tensor_mul(kl[:], PP[:], invSM[:])
            nc.vector.tensor_scalar_mul(kl[:], kl[:], float(1.0 / KL_TEMP))
            nc.vector.tensor_sub(kl[:], kl[:], lsem)
            nc.vector.tensor_add(kl[:], kl[:], lseT)

            # focal_row = (1-pt)^gamma * ce,  pt = exp(-ce)
            pt = vecp.tile([128, NT], F32, tag="pt")
            nc.scalar.activation(pt[:], ce[:], AF.Exp, scale=-1.0)
            c1 = vecp.tile([128, NT], F32, tag="c1")
            nc.vector.tensor_scalar(c1[:], pt[:], 0.5, None, op0=ALU.is_lt)
            c2 = vecp.tile([128, NT], F32, tag="c2")
            nc.vector.tensor_scalar(c2[:], pt[:], 0.2, None, op0=ALU.is_lt)
            # w = (1-pt)^gamma with gamma = 1 + 2*c1 + 2*c2:
            #   w = u * (u^2)^c1 * (u^2)^c2,  (u^2)^ci = 1 + ci*(u^2-1)
            u = vecp.tile([128, NT], F32, tag="u")
            nc.vector.tensor_scalar(u[:], pt[:], -1.0, 1.0,
                                    op0=ALU.mult, op1=ALU.add)
            t2 = vecp.tile([128, NT], F32, tag="t2")
            nc.vector.tensor_mul(t2[:], u[:], u[:])
            nc.vector.tensor_scalar(t2[:], t2[:], -1.0, None, op0=ALU.add)
            f1 = vecp.tile([128, NT], F32, tag="f1")
            nc.vector.tensor_mul(f1[:], c1[:], t2[:])
            nc.vector.tensor_scalar(f1[:], f1[:], 1.0, None, op0=ALU.add)
            f2 = vecp.tile([128, NT], F32, tag="f2")
            nc.vector.tensor_mul(f2[:], c2[:], t2[:])
            nc.vector.tensor_scalar(f2[:], f2[:], 1.0, None, op0=ALU.add)
            w = vecp.tile([128, NT], F32, tag="w")
            nc.vector.tensor_mul(w[:], u[:], f1[:])
            nc.vector.tensor_mul(w[:], w[:], f2[:])
            foc = vecp.tile([128, NT], F32, tag="foc")
            nc.vector.tensor_mul(foc[:], w[:], ce[:])

            # ---- reduce to partial sums, then across partitions via PE ----
            acc = vecp.tile([128, 8], F32, tag="acc")
            nc.vector.reduce_sum(acc[:, 0:1], kl[:], axis=AX.X)
            nc.vector.reduce_sum(acc[:, 1:2], ce[:], axis=AX.X)
            nc.vector.reduce_sum(acc[:, 2:3], adv[:], axis=AX.X)
            nc.vector.reduce_sum(acc[:, 3:4], foc[:], axis=AX.X)
            nc.vector.reduce_sum(acc[:, 4:5], trace_t[:], axis=AX.X)
            nc.vector.memset(acc[:, 5:8], 0.0)

            ps_f = psp.tile([8, 1], F32, tag="ps")
            nc.tensor.matmul(ps_f[:], acc[:], onesf_t[:],
                             start=True, stop=True)
            out_sb = vecp.tile([8, 1], F32, tag="out_sb")
            nc.scalar.copy(out_sb[:], ps_f[:])
            dma(out=res_d[:], in_=out_sb[:])
            dma(out=rsp_d[:], in_=rsp_t[:])

    nc.compile()
    return nc


_NC = None


def _get_nc():
    global _NC
    if _NC is None:
        _NC = _build_module()
    return _NC


def _prep_inputs(output, target, master_net_pred, feat_pooled,
                 feat_pooled_masked, output_adv, target_adv):
    o = np.asarray(output, dtype=np.float32).astype(ml_dtypes.bfloat16)
    m = np.asarray(master_net_pred,
                   dtype=np.float32).astype(ml_dtypes.bfloat16)
    a = np.asarray(output_adv, dtype=np.float32).astype(ml_dtypes.bfloat16)
    tg = np.asarray(target).astype(np.int64)
    ta = np.asarray(target_adv).astype(np.int64)
    f0 = np.asarray(feat_pooled, dtype=np.float32)
    f1 = np.asarray(feat_pooled_masked, dtype=np.float32)
    feats = np.concatenate([f0, f1], axis=0)  # [2B, D]

    in_maps = []
    for cc in range(NCORES):
        sl = slice(cc * RB, (cc + 1) * RB)
        # GLOBAL roll: preserves mod-8192 circulant distances, so the
        # half-open band covers each unordered pair exactly once fleet-wide.
        order = (np.arange(N2) + cc * RB) % N2
        ftc = np.ascontiguousarray(
            feats[order].T.astype(ml_dtypes.bfloat16))  # [D, 2B]
        in_maps.append({
            "o": np.ascontiguousarray(o[sl]),
            "m": np.ascontiguousarray(m[sl]),
            "a": np.ascontiguousarray(a[sl]),
            "tg": np.ascontiguousarray(
                tg[sl].reshape(NT, 128).T.astype(np.float16)),
            "ta": np.ascontiguousarray(
                ta[sl].reshape(NT, 128).T.astype(np.float16)),
            "ft": ftc,
        })
    return in_maps


def _combine(results):
    r = np.zeros(8, dtype=np.float64)
    rs = np.zeros(N2, dtype=np.float64)
    for cc, rr in enumerate(results):
        r += rr["res"].reshape(-1).astype(np.float64)
        rsp = rr["rsp"].astype(np.float64)        # [128, NQ*5]
        cspv = rr["csp"].astype(np.float64)       # [128, NBLK]
        rolled = cspv.T.reshape(-1).copy()        # rolled col 128*blk+p
        for k, Bq in enumerate(QBLKS):
            rows = slice(128 * Bq, 128 * Bq + 128)
            rolled[rows] += rsp[:, 3 * k:3 * k + 3].sum(axis=1)
        order = (np.arange(N2) + cc * RB) % N2
        rs[order] += rolled
    kl_mean = r[0] / (B * C)
    ce_mean = r[1] / B
    adv_mean = r[2] / B
    foc_mean = r[3] / B
    pos_sum = 2.0 * r[4] / NCE_TEMP          # sum of positive logits, all rows
    lse = np.log(rs)
    nce_mean = (lse.sum() - pos_sum) / N2
    loss = (KL_INTERP * KL_TEMP * KL_TEMP) * kl_mean \
        + (1.0 - KL_INTERP) * ce_mean + nce_mean + foc_mean + adv_mean
    return np.asarray([loss], dtype=np.float32)


def kernel(**inputs):
    in_maps = _prep_inputs(**inputs)
    out = run_bass_kernel_spmd(_get_nc(), in_maps,
                               core_ids=list(range(NCORES)))
    return _combine(out.results)


if __name__ == "__main__":
    rng = np.random.default_rng(0)
    ins = {
        "output": rng.standard_normal((B, C), dtype=np.float32),
        "target": rng.integers(0, C, size=(B,)),
        "master_net_pred": rng.standard_normal((B, C), dtype=np.float32),
        "feat_pooled": rng.standard_normal((B, D), dtype=np.float32),
        "feat_pooled_masked": rng.standard_normal((B, D), dtype=np.float32),
        "output_adv": rng.standard_normal((B, C), dtype=np.float32),
        "target_adv": rng.integers(0, C, size=(B,)),
    }
    print(kernel(**ins))


# revision 26
# speedup vs baseline: 2.5900x; 1.2467x over previous
"""Trainium2 Bass kernel for the combined loss (KL + CE + InfoNCE + focal + adv CE).

Strategy (8 NeuronCores, symmetric half-band InfoNCE):
  - o / master / o_adv sharded by rows (512/core), shipped bf16, packed as one
    [128, 3000] tile per 128-row group (one DMA each).
  - InfoNCE: feats = concat(feat_pooled, feat_pooled_masked) -> [8192, 256],
    transposed to [256, 8192] bf16 with a per-core GLOBAL column roll (core's
    1024 query rows land at rolled block positions {0..3, 32..35}).  Exploits
    G = G^T: each query block computes only its contiguous 33-block extent
    [self, +1..+32] of the circulant band, with triangular masks on the self
    and +32 blocks so every unordered pair is computed exactly once
    fleet-wide.  Each exp(l_ij) serves row i via the ScalarE accumulate (row
    sums) and row j via column sums: the 8 query blocks' exp tiles are first
    accumulated into one [128, 8192] rolled-column tile on the DVE, then a
    single pass of 64 rank-reducing matmuls (exp block as lhsT x ones column)
    extracts all block column sums.  Per-core partial row/col sums + positive
    trace go to the host, which assembles the 8192 row sums, takes the log,
    and averages (the only cross-core reduction).
  - Normalization: squares on DVE, column sums-of-squares via a bf16
    ones-matmul, rn = exp(-0.5*ln(s)) on ScalarE, normalize mult on DVE.
  - The activation-table map is restricted so Exp and Ln both resolve to
    natural_log_exp_and_others: one table load, no Ln<->Exp thrashing.
"""

import numpy as np
import ml_dtypes

import concourse.bacc as bacc
import concourse.tile as tile
from concourse import mybir
from concourse.bass_utils import run_bass_kernel_spmd

F32 = mybir.dt.float32
BF16 = mybir.dt.bfloat16
FP16 = mybir.dt.float16
AF = mybir.ActivationFunctionType
ALU = mybir.AluOpType
AX = mybir.AxisListType

NCORES = 8
B, C, D = 4096, 1000, 256
RB = B // NCORES          # 512 rows of the [B, C] tensors per core
NT = RB // 128            # 4 row-tiles per core
N2 = 2 * B                # 8192 infoNCE rows
NBLK = N2 // 128          # 64 column blocks
QBLKS = [0, 1, 2, 3, 32, 33, 34, 35]   # rolled block positions of queries
NQ = len(QBLKS)
ESW = 33 * 128            # extent: self block + 32 band blocks = 4224 cols
KCH = 1024                # gram / psum chunk width
SUB = 512                 # matmul moving free dim
NSLOT = 5                 # rowsum slots per query block (4 chunks + tail)

KL_TEMP = 4.0
KL_INTERP = 0.5
NCE_TEMP = 0.07
NEG_BIG = -1.0e9

_orig_gat = bacc.get_activation_tables


def _pinned_tables(arch):
    """Keep canonical set order/ids but make Exp and Ln resolve only to
    natural_log_exp_and_others so the planner never reloads tables."""
    t = _orig_gat(arch)
    if "natural_log_exp_and_others" in t:
        nle = t["natural_log_exp_and_others"]
        if AF.Exp in nle and AF.Ln in nle:
            for name, s in t.items():
                if name != "natural_log_exp_and_others":
                    s.discard(AF.Exp)
                    s.discard(AF.Ln)
    return t


def _col_runs(start, e0, e1):
    """Contiguous rolled-column runs covering extent offsets [e0, e1)."""
    runs = []
    e = e0
    while e < e1:
        rs = (start + e) % N2
        ln = min(e1 - e, N2 - rs)
        runs.append((rs, e, ln))
        e += ln
    return runs


def _build_module():
    bacc.get_activation_tables = _pinned_tables
    nc = bacc.Bacc("TRN2", target_bir_lowering=False, debug=False)

    oma_d = nc.dram_tensor("oma", [RB, 3 * C], BF16, kind="ExternalInput")
    ft_d = nc.dram_tensor("ft", [256, N2], BF16, kind="ExternalInput")
    fp_d = nc.dram_tensor("fp", [128, C + 8], FP16, kind="ExternalInput")
    res_d = nc.dram_tensor("res", [8, 1], F32, kind="ExternalOutput")
    rsp_d = nc.dram_tensor("rsp", [128, NQ * NSLOT], F32,
                           kind="ExternalOutput")
    # csp[p, blk] = partial row sum for rolled row 128*blk+p
    csp_d = nc.dram_tensor("csp", [128, NBLK], F32, kind="ExternalOutput")

    r_idx = np.arange(128)[:, None]
    c_idx = np.arange(128)[None, :]
    # matmul adds lhsT^T; bake the transpose into the constants.
    cb16 = np.concatenate([
        np.eye(128),                          # identb
        np.ones((128, 128)),                  # onesb
        NEG_BIG * (r_idx > c_idx),            # keep c<=r   (strict mask^T)
        NEG_BIG * (r_idx >= c_idx),           # keep c<r    (incl mask^T)
    ], axis=1).astype(ml_dtypes.bfloat16)
    cf32 = np.concatenate([np.eye(128), np.ones((128, 1))],
                          axis=1).astype(np.float32)
    cb16_d = nc.inline_tensor(cb16, "cb16")
    cf32_d = nc.inline_tensor(cf32, "cf32")

    from contextlib import ExitStack
    with tile.TileContext(nc) as tc:
        with (
            tc.tile_pool(name="persist", bufs=1) as persist,
            tc.tile_pool(name="io", bufs=2) as iop,
            tc.tile_pool(name="scr", bufs=4) as scrp,
            tc.tile_pool(name="vec", bufs=1) as vecp,
            tc.tile_pool(name="ps", bufs=4, space="PSUM") as psp,
            ExitStack() as late_ctx,
        ):
            dma = nc.default_dma_engine.dma_start

            cb_t = persist.tile([128, 512], BF16, tag="cb16")
            dma(out=cb_t[:], in_=cb16_d[:])
            identb_t = cb_t[:, 0:128]
            onesb_t = cb_t[:, 128:256]
            masks_t = cb_t[:, 256:384]
            maski_t = cb_t[:, 384:512]
            cf_t = persist.tile([128, 129], F32, tag="cf32")
            dma(out=cf_t[:], in_=cf32_d[:])
            ident_t = cf_t[:, 0:128]
            onesf_t = cf_t[:, 128:129]
            fp_t = persist.tile([128, C + 8], FP16, tag="fp")
            dma(out=fp_t[:], in_=fp_d[:])
            iota_t = fp_t[:, 0:C]
            tg_t = fp_t[:, C:C + 4]
            ta_t = fp_t[:, C + 4:C + 8]

            hn0 = persist.tile([128, N2], BF16, tag="hn0")
            hn1 = persist.tile([128, N2], BF16, tag="hn1")
            essum = persist.tile([128, N2], BF16, tag="essum")
            rsp_t = persist.tile([128, NQ * NSLOT], F32, tag="rsp")
            trace_t = vecp.tile([128, 4], F32, tag="trace")

            # cekl per-row stat slots; the four lse stats share one tile so a
            # single Ln covers them
            stats = vecp.tile([128, 16], F32, tag="stats")
            PP = vecp.tile([128, NT], F32, tag="PP")
            GO = vecp.tile([128, NT], F32, tag="GO")
            GA = vecp.tile([128, NT], F32, tag="GA")

            def cekl_tile(t):
                rsl = slice(t * 128, (t + 1) * 128)
                oma_t = iop.tile([128, 3 * C], BF16, tag="oma")
                dma(out=oma_t[:], in_=oma_d[rsl, :])
                o_t = oma_t[:, 0:C]
                m_t = oma_t[:, C:2 * C]
                a_t = oma_t[:, 2 * C:3 * C]

                e1 = scrp.tile([128, C], BF16, tag="scr1000")
                nc.scalar.activation(e1[:], o_t, AF.Exp, scale=1.0,
                                     accum_out=stats[:, t:t + 1])
                e2 = scrp.tile([128, C], BF16, tag="scr1000")
                nc.scalar.activation(e2[:], o_t, AF.Exp,
                                     scale=float(1.0 / KL_TEMP),
                                     accum_out=stats[:, 4 + t:5 + t])
                em_t = iop.tile([128, C], BF16, tag="em")
                nc.scalar.activation(em_t[:], m_t, AF.Exp,
                                     scale=float(1.0 / KL_TEMP),
                                     accum_out=stats[:, 8 + t:9 + t])
                e3 = scrp.tile([128, C], BF16, tag="scr1000")
                nc.scalar.activation(e3[:], a_t, AF.Exp, scale=1.0,
                                     accum_out=stats[:, 12 + t:13 + t])

                d_t = iop.tile([128, C], BF16, tag="d")
                nc.vector.tensor_sub(d_t[:], m_t, o_t)
                pr = scrp.tile([128, C], BF16, tag="scr1000")
                nc.vector.scalar_tensor_tensor(
                    out=pr[:], in0=d_t[:], scalar=1.0, in1=em_t[:],
                    op0=ALU.mult, op1=ALU.mult, accum_out=PP[:, t:t + 1])
                g1 = scrp.tile([128, C], BF16, tag="scr1000")
                nc.vector.scalar_tensor_tensor(
                    out=g1[:], in0=iota_t, scalar=tg_t[:, t:t + 1],
                    in1=o_t, op0=ALU.is_equal, op1=ALU.mult,
                    accum_out=GO[:, t:t + 1])
                g2 = scrp.tile([128, C], BF16, tag="scr1000")
                nc.vector.scalar_tensor_tensor(
                    out=g2[:], in0=iota_t, scalar=ta_t[:, t:t + 1],
                    in1=a_t, op0=ALU.is_equal, op1=ALU.mult,
                    accum_out=GA[:, t:t + 1])

            # ---- prologue: cekl tiles 0/1 keep ACT busy during feature DMA
            cekl_tile(0)
            cekl_tile(1)

            raw_pool_cm = tc.tile_pool(name="raw", bufs=1)
            rawp = raw_pool_cm.__enter__()
            h0 = rawp.tile([128, N2], BF16, tag="h0")
            h1 = rawp.tile([128, N2], BF16, tag="h1")
            rn = rawp.tile([128, N2], BF16, tag="rn")
            dma(out=h0[:], in_=ft_d[0:128, :])
            dma(out=h1[:], in_=ft_d[128:256, :])

            # ---- normalization pipeline, chunked; squares are staged in the
            # future hn tiles (overwritten by the normalize mult right after
            # the ones-matmul consumes them)
            NCH = N2 // KCH
            for ch in range(NCH):
                sl = slice(ch * KCH, (ch + 1) * KCH)
                nc.vector.tensor_mul(hn0[:, sl], h0[:, sl], h0[:, sl])
                nc.vector.tensor_mul(hn1[:, sl], h1[:, sl], h1[:, sl])
                ps_n = psp.tile([128, KCH], F32, tag="ps")
                for half, sq in ((0, hn0), (1, hn1)):
                    for s in range(KCH // SUB):
                        c0 = ch * KCH + s * SUB
                        osl = slice(s * SUB, (s + 1) * SUB)
                        nc.tensor.matmul(ps_n[:, osl], onesb_t,
                                         sq[:, c0:c0 + SUB],
                                         start=(half == 0), stop=(half == 1))
                nc.scalar.activation(rn[:, sl], ps_n[:], AF.Ln)
                nc.scalar.activation(rn[:, sl], rn[:, sl], AF.Exp, scale=-0.5)
                nc.vector.tensor_mul(hn0[:, sl], h0[:, sl], rn[:, sl])
                nc.vector.tensor_mul(hn1[:, sl], h1[:, sl], rn[:, sl])

            cekl_tile(2)
            cekl_tile(3)

            # raw features / rn are dead; release before the es pool opens.
            raw_pool_cm.__exit__(None, None, None)
            esp = late_ctx.enter_context(tc.tile_pool(name="es", bufs=3))

            # ---- symmetric half-band InfoNCE ----
            CHUNKS = [(0, KCH), (KCH, KCH), (2 * KCH, KCH), (3 * KCH, KCH),
                      (4 * KCH, 128)]
            # ct wave 1: blocks untouched by the second-half query extents
            wave1 = [b for b in range(NBLK)
                     if not any((b - Bq) % NBLK <= 32 for Bq in QBLKS[4:])]

            def ct_extract(blocks):
                # shares the "ps" psum slots with the gram chunks
                ct_ps = psp.tile([128, len(blocks)], F32, tag="ps")
                for i, bk in enumerate(blocks):
                    nc.tensor.matmul(ct_ps[:, i:i + 1],
                                     essum[:, bk * 128:(bk + 1) * 128],
                                     onesb_t[:, 0:1], start=True, stop=True)
                ct_sb = vecp.tile([128, len(blocks)], F32,
                                  tag=f"ct_sb{blocks[0]}")
                nc.scalar.copy(ct_sb[:], ct_ps[:])
                i = 0
                while i < len(blocks):
                    j = i
                    while j + 1 < len(blocks) and blocks[j + 1] == blocks[j] + 1:
                        j += 1
                    dma(out=csp_d[:, blocks[i]:blocks[j] + 1],
                        in_=ct_sb[:, i:j + 1])
                    i = j + 1

            written = np.zeros(NBLK, dtype=bool)
            for qi, Bq in enumerate(QBLKS):
                cb = 128 * Bq
                lhs0 = hn0[:, cb:cb + 128]
                lhs1 = hn1[:, cb:cb + 128]
                es_t = esp.tile([128, ESW], BF16, tag="es")
                for ci, (base0, cw) in enumerate(CHUNKS):
                    ps_k = psp.tile([128, cw], F32, tag="ps")
                    for half, hn, lhsT in ((0, hn0, lhs0), (1, hn1, lhs1)):
                        for s in range((cw + SUB - 1) // SUB):
                            b0 = base0 + s * SUB
                            b1 = min(base0 + cw, b0 + SUB)
                            for (rs_c, e, ln) in _col_runs(cb, b0, b1):
                                off = e - base0
                                nc.tensor.matmul(
                                    ps_k[:, off:off + ln], lhsT,
                                    hn[:, rs_c:rs_c + ln],
                                    start=(half == 0), stop=(half == 1))
                        if half == 0 and ci == 0:
                            # self block: keep strict lower triangle
                            nc.tensor.matmul(ps_k[:, 0:128], maski_t,
                                             identb_t, start=False,
                                             stop=False,
                                             skip_group_check=True)
                        if half == 0 and ci == 4:
                            # +32 block: tie-break mask on the pair diagonal
                            mk = masks_t if qi < 4 else maski_t
                            nc.tensor.matmul(ps_k[:, 0:128], mk,
                                             identb_t, start=False,
                                             stop=False,
                                             skip_group_check=True)
                    nc.scalar.activation(
                        es_t[:, base0:base0 + cw], ps_k[:], AF.Exp,
                        scale=float(1.0 / NCE_TEMP),
                        accum_out=rsp_t[:, qi * NSLOT + ci:qi * NSLOT + ci + 1])
                    if ci == 4 and qi < 4:
                        tr_scr = scrp.tile([128, 128], F32, tag="trscr")
                        nc.vector.scalar_tensor_tensor(
                            out=tr_scr[:], in0=ps_k[:, 0:128], scalar=1.0,
                            in1=ident_t, op0=ALU.mult, op1=ALU.mult,
                            accum_out=trace_t[:, qi:qi + 1])
                # accumulate exp tile into the rolled-column accumulator;
                # first touch of a block is a copy (no upfront memset)
                ext_blocks = [(Bq + k) % NBLK for k in range(33)]
                seg = 0
                while seg < 33:
                    new = written[ext_blocks[seg]] == False  # noqa: E712
                    end = seg
                    while end + 1 < 33 and \
                            (written[ext_blocks[end + 1]] == False) == new:
                        end += 1
                    for (rs_c, e, ln) in _col_runs(cb, seg * 128,
                                                   (end + 1) * 128):
                        if new:
                            nc.vector.tensor_copy(essum[:, rs_c:rs_c + ln],
                                                  es_t[:, e:e + ln])
                        else:
                            nc.vector.tensor_add(essum[:, rs_c:rs_c + ln],
                                                 essum[:, rs_c:rs_c + ln],
                                                 es_t[:, e:e + ln])
                    for b in ext_blocks[seg:end + 1]:
                        written[b] = True
                    seg = end + 1
                if qi == 3:
                    ct_extract(wave1)

            ct_extract([b for b in range(NBLK) if b not in wave1])

            # ---- epilogue on [128, NT] stat vectors ----
            lse_all = vecp.tile([128, 16], F32, tag="lse_all")
            nc.scalar.activation(lse_all[:], stats[:], AF.Ln)
            lse1 = lse_all[:, 0:4]
            lseT = lse_all[:, 4:8]
            lsem = lse_all[:, 8:12]
            lsea = lse_all[:, 12:16]

            ce = vecp.tile([128, NT], F32, tag="ce")
            nc.vector.tensor_sub(ce[:], lse1, GO[:])
            adv = vecp.tile([128, NT], F32, tag="adv")
            nc.vector.tensor_sub(adv[:], lsea, GA[:])

            # kl_row = PP/(T*SM) - lsem + lseT
            invSM = vecp.tile([128, NT], F32, tag="invSM")
            nc.vector.reciprocal(invSM[:], stats[:, 8:12])
            kl = vecp.tile([128, NT], F32, tag="kl")
            nc.vector.tensor_mul(kl[:], PP[:], invSM[:])
            nc.vector.tensor_scalar_mul(kl[:], kl[:], float(1.0 / KL_TEMP))
            nc.vector.tensor_sub(kl[:], kl[:], lsem)
            nc.vector.tensor_add(kl[:], kl[:], lseT)

            # focal_row = (1-pt)^gamma * ce,  pt = exp(-ce)
            pt = vecp.tile([128, NT], F32, tag="pt")
            nc.scalar.activation(pt[:], ce[:], AF.Exp, scale=-1.0)
            c1 = vecp.tile([128, NT], F32, tag="c1")
            nc.vector.tensor_scalar(c1[:], pt[:], 0.5, None, op0=ALU.is_lt)
            c2 = vecp.tile([128, NT], F32, tag="c2")
            nc.vector.tensor_scalar(c2[:], pt[:], 0.2, None, op0=ALU.is_lt)
            # w = (1-pt)^gamma with gamma = 1 + 2*c1 + 2*c2:
            #   w = u * (u^2)^c1 * (u^2)^c2,  (u^2)^ci = 1 + ci*(u^2-1)
            u = vecp.tile([128, NT], F32, tag="u")
            nc.vector.tensor_scalar(u[:], pt[:], -1.0, 1.0,
                                    op0=ALU.mult, op1=ALU.add)
            t2 = vecp.tile([128, NT], F32, tag="t2")
            nc.vector.tensor_mul(t2[:], u[:], u[:])
            nc.vector.tensor_scalar(t2[:], t2[:], -1.0, None, op0=ALU.add)
            f1 = vecp.tile([128, NT], F32, tag="f1")
            nc.vector.tensor_mul(f1[:], c1[:], t2[:])
            nc.vector.tensor_scalar(f1[:], f1[:], 1.0, None, op0=ALU.add)
            f2 = vecp.tile([128, NT], F32, tag="f2")
            nc.vector.tensor_mul(f2[:], c2[:], t2[:])
            nc.vector.tensor_scalar(f2[:], f2[:], 1.0, None, op0=ALU.add)
            w = vecp.tile([128, NT], F32, tag="w")
            nc.vector.tensor_mul(w[:], u[:], f1[:])
            nc.vector.tensor_mul(w[:], w[:], f2[:])
            foc = vecp.tile([128, NT], F32, tag="foc")
            nc.vector.tensor_mul(foc[:], w[:], ce[:])

            # ---- reduce to partial sums, then across partitions via PE ----
            acc = vecp.tile([128, 8], F32, tag="acc")
            nc.vector.reduce_sum(acc[:, 0:1], kl[:], axis=AX.X)
            nc.vector.reduce_sum(acc[:, 1:2], ce[:], axis=AX.X)
            nc.vector.reduce_sum(acc[:, 2:3], adv[:], axis=AX.X)
            nc.vector.reduce_sum(acc[:, 3:4], foc[:], axis=AX.X)
            nc.vector.reduce_sum(acc[:, 4:5], trace_t[:], axis=AX.X)
            nc.vector.memset(acc[:, 5:8], 0.0)

            ps_f = psp.tile([8, 1], F32, tag="ps")
            nc.tensor.matmul(ps_f[:], acc[:], onesf_t,
                             start=True, stop=True)
            out_sb = vecp.tile([8, 1], F32, tag="out_sb")
            nc.scalar.copy(out_sb[:], ps_f[:])
            dma(out=res_d[:], in_=out_sb[:])
            dma(out=rsp_d[:], in_=rsp_t[:])

    nc.compile()
    return nc


_NC = None


def _get_nc():
    global _NC
    if _NC is None:
        _NC = _build_module()
    return _NC


def _prep_inputs(output, target, master_net_pred, feat_pooled,
                 feat_pooled_masked, output_adv, target_adv):
    o = np.asarray(output, dtype=np.float32)
    m = np.asarray(master_net_pred, dtype=np.float32)
    a = np.asarray(output_adv, dtype=np.float32)
    oma = np.concatenate([o.reshape(B // 128, 128, C),
                          m.reshape(B // 128, 128, C),
                          a.reshape(B // 128, 128, C)],
                         axis=2).astype(ml_dtypes.bfloat16)  # [B/128,128,3C]
    tg = np.asarray(target).astype(np.int64)
    ta = np.asarray(target_adv).astype(np.int64)
    f0 = np.asarray(feat_pooled, dtype=np.float32)
    f1 = np.asarray(feat_pooled_masked, dtype=np.float32)
    feats = np.concatenate([f0, f1], axis=0)  # [2B, D]
    iota = np.tile(np.arange(C, dtype=np.float16), (128, 1))

    in_maps = []
    for cc in range(NCORES):
        # GLOBAL roll: preserves mod-8192 circulant distances, so the
        # half-open band covers each unordered pair exactly once fleet-wide.
        order = (np.arange(N2) + cc * RB) % N2
        ftc = np.ascontiguousarray(
            feats[order].T.astype(ml_dtypes.bfloat16))  # [D, 2B]
        tgc = tg[cc * RB:(cc + 1) * RB].reshape(NT, 128).T
        tac = ta[cc * RB:(cc + 1) * RB].reshape(NT, 128).T
        fp = np.concatenate([iota, tgc.astype(np.float16),
                             tac.astype(np.float16)], axis=1)
        in_maps.append({
            "oma": np.ascontiguousarray(
                oma[4 * cc:4 * cc + 4].reshape(RB, 3 * C)),
            "ft": ftc,
            "fp": np.ascontiguousarray(fp),
        })
    return in_maps


def _combine(results):
    r = np.zeros(8, dtype=np.float64)
    rs = np.zeros(N2, dtype=np.float64)
    for cc, rr in enumerate(results):
        r += rr["res"].reshape(-1).astype(np.float64)
        rsp = rr["rsp"].astype(np.float64)        # [128, NQ*NSLOT]
        cspv = rr["csp"].astype(np.float64)       # [128, NBLK]
        rolled = cspv.T.reshape(-1).copy()        # rolled col 128*blk+p
        for k, Bq in enumerate(QBLKS):
            rows = slice(128 * Bq, 128 * Bq + 128)
            rolled[rows] += rsp[:, NSLOT * k:NSLOT * (k + 1)].sum(axis=1)
        order = (np.arange(N2) + cc * RB) % N2
        rs[order] += rolled
    kl_mean = r[0] / (B * C)
    ce_mean = r[1] / B
    adv_mean = r[2] / B
    foc_mean = r[3] / B
    pos_sum = 2.0 * r[4] / NCE_TEMP          # sum of positive logits, all rows
    lse = np.log(rs)
    nce_mean = (lse.sum() - pos_sum) / N2
    loss = (KL_INTERP * KL_TEMP * KL_TEMP) * kl_mean \
        + (1.0 - KL_INTERP) * ce_mean + nce_mean + foc_mean + adv_mean
    return np.asarray([loss], dtype=np.float32)


def kernel(**inputs):
    in_maps = _prep_inputs(**inputs)
    out = run_bass_kernel_spmd(_get_nc(), in_maps,
                               core_ids=list(range(NCORES)))
    return _combine(out.results)


if __name__ == "__main__":
    rng = np.random.default_rng(0)
    ins = {
        "output": rng.standard_normal((B, C), dtype=np.float32),
        "target": rng.integers(0, C, size=(B,)),
        "master_net_pred": rng.standard_normal((B, C), dtype=np.float32),
        "feat_pooled": rng.standard_normal((B, D), dtype=np.float32),
        "feat_pooled_masked": rng.standard_normal((B, D), dtype=np.float32),
        "output_adv": rng.standard_normal((B, C), dtype=np.float32),
        "target_adv": rng.integers(0, C, size=(B,)),
    }
    print(kernel(**ins))
